# revision 7
# baseline (speedup 1.0000x reference)
"""Trainium2 Bass kernel for nn_JointNet_23785528885377 (retrieval_knn).

Math note: the reference computes nn_idx = argmin(d2, axis=1) over the full
NxN squared-distance matrix but only consumes row 0 of the gathered
neighbors (exp_neighbor = exp(neigh[0]) = exp(f[nn_idx[0]])). Coords are
ints < 100, so d2 is exact integer arithmetic in fp32, d2[0,0] == 0 is the
global minimum of row 0, and argmin tie-breaks to the lowest index =>
nn_idx[0] == 0 for ANY valid input. Hence exp_neighbor == exp(relu(x[0,:]))
and the whole cdist+argmin is dead code. Per cloud:

    f      = relu(x)                               [N,C]
    rowmax = max_c f                               [N]
    gamma  = max_c(f * exp(f) * exp(-f0)) / rowmax [N]   (f0 = relu(x[0,:]))
    out    = gamma / ||gamma||_2

Further algebra used by the v3 kernel (valid whenever a row has at least
one nonnegative channel, which holds for this data — all-negative rows
give NaN in the reference anyway via 0/0):

    relu can be dropped inside the max: for x<0 the term x*exp(x)*e0inv
    is negative while the relu'd term is 0, so it never wins the max.
    rowmax = max_c x = ln(max_c exp(x))  (exp monotone), so one dual
    reduce over [exp(x) | x*e0inv*exp(x)] yields both row stats.
    1/sqrt(s) = exp(-0.5*ln(s)) keeps ACT on two tables (Exp, Ln).

Sharding: one cloud per core (B=2 clouds; cores 2-7 run the same SPMD
program on duplicate data and are ignored). A cross-core AllReduce was
measured at ~56us active time on this runtime — far more than the whole
kernel — so the row-sharded 8-core variant loses; data-parallel it is.
"""

import os

import numpy as np
from contextlib import ExitStack

import concourse.bass as bass
import concourse.bacc as bacc
import concourse.tile as tile
from concourse import mybir
from concourse.bass_utils import run_bass_kernel_spmd

B, N, C = 2, 12288, 32
P = 128
NCORES = 8

AF = mybir.ActivationFunctionType
F32 = mybir.dt.float32
F16 = mybir.dt.float16


def build_v3(n_chunks=4, use_f16=True, dual_queue=True, tot_eng="pe",
             t2_eng="gpsimd", half_first=True):
    """Optimized single-cloud kernel. Empirical constraints of this stack:
    - gpsimd compute silently corrupts when reading ACT-produced tiles and
      hard-crashes on stride-0 APs; safe with DMA/DVE-produced inputs.
    - tensor_tensor_reduce / AP-scalar tensor_scalar don't lower (symbolic
      AP assert); gpsimd free-axis reduces unsupported (axis C only).
    - Only one ACT table stays resident: every Exp<->Sqrt/Ln switch costs
      1.28us, so the kernel uses Exp for the chunks and one Sqrt, whose
      table load is pinned (via a data dependency on the last exp) to
      overlap the DVE drain.
    - DVE reduces run ~1 elem/cycle/lane regardless of dtype; f16 only
      speeds tensor_tensor (2x). DVE is the bottleneck: per chunk it runs
      prod-mul (f16) + 2 row-max reduces; t2 = x*e0inv runs on gpsimd.

    Epilogue is hop-minimized: one DVE block (1/rowmax, gamma, gamma^2,
    row-sum), cross-partition sum via PE (or gpsimd axis-C reduce), DVE
    reciprocal, ACT sqrt, PE broadcast, DVE final scale, DMA out.
    """
    T = N // P
    assert T % n_chunks == 0
    TCH = T // n_chunks

    nc = bacc.Bacc("TRN2", target_bir_lowering=False, debug=False)
    x = nc.dram_tensor("x", [N, C], F32, kind="ExternalInput")
    r0 = nc.dram_tensor("row0", [1, C], F32, kind="ExternalInput")
    y = nc.dram_tensor("y", [N], F32, kind="ExternalOutput")

    xv = x.rearrange("(p t) c -> p t c", p=P)  # [128, 96, 32]
    yv = y.rearrange("(p t) -> p t", p=P)      # [128, 96]

    EDT = F16 if use_f16 else F32

    with tile.TileContext(nc) as tc, ExitStack() as ctx:
        pool = ctx.enter_context(tc.tile_pool(name="main", bufs=1))
        ch = ctx.enter_context(tc.tile_pool(name="chunks", bufs=n_chunks))
        psum = ctx.enter_context(tc.tile_pool(name="psum", bufs=1, space="PSUM"))

        # r0 load first (1 descriptor, lands fast), then all chunk DMA
        # triggers before any ACT op so no trigger queues behind the Exp
        # table load.
        r0row = pool.tile([1, C], F32)
        nc.sync.dma_start(out=r0row[:], in_=r0[0:1, :])
        xts = []
        for j in range(n_chunks):
            sl = slice(j * TCH, (j + 1) * TCH)
            xt = ch.tile([P, TCH, C], F32, tag=f"xt{j}")
            if j == 0 and half_first and dual_queue:
                # split chunk 0 across both queues so compute ramps sooner
                nc.sync.dma_start(out=xt[0:64], in_=xv[0:64, sl, :])
                nc.scalar.dma_start(out=xt[64:128], in_=xv[64:128, sl, :])
            else:
                dmae = nc.scalar if (dual_queue and j % 2 == 1) else nc.sync
                dmae.dma_start(out=xt[:], in_=xv[:, sl, :])
            xts.append(xt)

        # e0inv = exp(-relu(row0)), broadcast to [P,TCH,C] via PE + DVE copy
        f0 = pool.tile([1, C], F32)
        nc.vector.tensor_scalar_max(f0[:], r0row[:], 0.0)
        e0row = pool.tile([1, C], F32)
        nc.scalar.activation(out=e0row[:], in_=f0[:], func=AF.Exp, scale=-1.0)
        onesr = pool.tile([1, P], F32)
        nc.vector.memset(onesr[:], 1.0)
        ones = pool.tile([P, 1], F32)
        nc.vector.memset(ones[:], 1.0)
        e0psum = psum.tile([P, C], F32)
        nc.tensor.matmul(e0psum[:], onesr[:], e0row[:])
        e0mat = pool.tile([P, TCH, C], F32)
        e0p = e0psum[:]
        nc.vector.tensor_copy(
            e0mat[:],
            bass.AP(tensor=e0p.tensor, offset=e0p.offset,
                    ap=[e0p.ap[0], [0, TCH], e0p.ap[1]]),
        )
        t2e = getattr(nc, t2_eng)

        # [:, j, 0, :] = rowmax(x), [:, j, 1, :] = rowmax(x*e0inv*exp(x))
        mm = pool.tile([P, n_chunks, 2, TCH], F32)

        exs = []
        for j in range(n_chunks):
            xt = xts[j]
            ex = ch.tile([P, TCH, C], EDT, tag=f"ex{j}")
            nc.scalar.activation(out=ex[:], in_=xt[:], func=AF.Exp)
            exs.append(ex)
            t2 = ch.tile([P, TCH, C], EDT, tag=f"t2{j}")
            t2e.tensor_mul(t2[:], xt[:], e0mat[:])
            prod = ch.tile([P, TCH, C], EDT, tag=f"prod{j}")
            nc.vector.tensor_mul(prod[:], ex[:], t2[:])
            nc.vector.reduce_max(out=mm[:, j, 1, :], in_=prod[:],
                                 axis=mybir.AxisListType.X)
            nc.vector.reduce_max(out=mm[:, j, 0, :], in_=xt[:],
                                 axis=mybir.AxisListType.X)

        # prefetch the Sqrt table; the read of ex[-1] pins it after the
        # last exp so the load overlaps the DVE drain of chunks 2-3.
        sqdummy = pool.tile([1, 1], F32)
        nc.scalar.activation(out=sqdummy[:], in_=exs[-1][0:1, 0, 0:1],
                             func=AF.Sqrt)

        # ---- epilogue: one DVE block, then the norm scalar chain ----
        rinv = pool.tile([P, T], F32)
        nc.vector.reciprocal(out=rinv[:], in_=mm[:, :, 0, :])
        gam = pool.tile([P, T], F32)
        nc.vector.tensor_mul(gam[:], mm[:, :, 1, :], rinv[:])
        sq = pool.tile([P, T], F32)
        nc.vector.tensor_mul(sq[:], gam[:], gam[:])
        ssq = pool.tile([P, 1], F32)
        nc.vector.reduce_sum(out=ssq[:], in_=sq[:], axis=mybir.AxisListType.X)

        rec = pool.tile([1, 1], F32)
        if tot_eng == "gpsimd":
            tot_sb = pool.tile([1, 1], F32)
            nc.gpsimd.tensor_reduce(out=tot_sb[:], in_=ssq[:],
                                    axis=mybir.AxisListType.C,
                                    op=mybir.AluOpType.add)
            nc.vector.reciprocal(out=rec[:], in_=tot_sb[:])
        else:
            tot = psum.tile([1, 1], F32)
            nc.tensor.matmul(tot[:], ssq[:], ones[:])
            nc.vector.reciprocal(out=rec[:], in_=tot[:])
        rstd = pool.tile([1, 1], F32)
        nc.scalar.activation(out=rstd[:], in_=rec[:], func=AF.Sqrt)
        bc = psum.tile([P, 1], F32)
        nc.tensor.matmul(bc[:], onesr[:], rstd[:])
        bcs = pool.tile([P, 1], F32)
        nc.vector.tensor_copy(bcs[:], bc[:])
        outt = pool.tile([P, T], F32)
        bcsap = bcs[:]
        nc.vector.tensor_mul(
            outt[:], gam[:],
            bass.AP(tensor=bcsap.tensor, offset=bcsap.offset,
                    ap=[bcsap.ap[0], [0, T]]),
        )
        nc.sync.dma_start(out=yv[:], in_=outt[:])

    nc.compile()
    return nc


def build_nc(n_rows=N, n_chunks=4, bufs=2):
    """v1 baseline (kept as fallback): relu/exp on ACT, muls+reduces on DVE."""
    T = n_rows // P
    TCH = T // n_chunks

    nc = bacc.Bacc("TRN2", target_bir_lowering=False, debug=False)
    x = nc.dram_tensor("x", [n_rows, C], F32, kind="ExternalInput")
    r0 = nc.dram_tensor("row0", [1, C], F32, kind="ExternalInput")
    y = nc.dram_tensor("y", [n_rows], F32, kind="ExternalOutput")

    xv = x.rearrange("(p t) c -> p t c", p=P)
    yv = y.rearrange("(p t) -> p t", p=P)

    with tile.TileContext(nc) as tc, ExitStack() as ctx:
        pool = ctx.enter_context(tc.tile_pool(name="main", bufs=1))
        ch = ctx.enter_context(tc.tile_pool(name="chunks", bufs=bufs))
        psum = ctx.enter_context(tc.tile_pool(name="psum", bufs=1, space="PSUM"))

        r0rep = pool.tile([P, C], F32)
        r0ap = r0[0, :]
        nc.sync.dma_start(
            out=r0rep[:],
            in_=bass.AP(tensor=r0ap.tensor, offset=r0ap.offset,
                        ap=[[0, P]] + list(r0ap.ap)),
        )
        e0 = pool.tile([P, C], F32)
        nc.scalar.activation(out=e0[:], in_=r0rep[:], func=AF.Relu)
        nc.scalar.activation(out=e0[:], in_=e0[:], func=AF.Exp, scale=-1.0)
        e0ap = e0[:]

        gam = pool.tile([P, T, 1], F32)
        for j in range(n_chunks):
            sl = slice(j * TCH, (j + 1) * TCH)
            xt = ch.tile([P, TCH, C], F32, tag="xt")
            nc.sync.dma_start(out=xt[:], in_=xv[:, sl, :])
            f = ch.tile([P, TCH, C], F32, tag="f")
            nc.scalar.activation(out=f[:], in_=xt[:], func=AF.Relu)
            rmax = ch.tile([P, TCH, 1], F32, tag="rmax")
            nc.vector.reduce_max(out=rmax[:], in_=f[:], axis=mybir.AxisListType.X)
            ex = ch.tile([P, TCH, C], F32, tag="ex")
            nc.scalar.activation(out=ex[:], in_=f[:], func=AF.Exp)
            nc.vector.tensor_mul(ex[:], ex[:], f[:])
            e0b = bass.AP(tensor=e0ap.tensor, offset=e0ap.offset,
                          ap=[e0ap.ap[0], [0, TCH], e0ap.ap[1]])
            nc.vector.tensor_mul(ex[:], ex[:], e0b)
            m = ch.tile([P, TCH, 1], F32, tag="m")
            nc.vector.reduce_max(out=m[:], in_=ex[:], axis=mybir.AxisListType.X)
            rinv = ch.tile([P, TCH, 1], F32, tag="rinv")
            nc.vector.reciprocal(out=rinv[:], in_=rmax[:])
            nc.vector.tensor_mul(gam[:, sl, :], m[:], rinv[:])

        sq = pool.tile([P, T, 1], F32)
        ssq = pool.tile([P, 1], F32)
        nc.scalar.activation(out=sq[:], in_=gam[:], func=AF.Square,
                             accum_out=ssq[:])
        ones = pool.tile([P, 1], F32)
        nc.vector.memset(ones[:], 1.0)
        onesr = pool.tile([1, P], F32)
        nc.vector.memset(onesr[:], 1.0)
        tot = psum.tile([1, 1], F32)
        nc.tensor.matmul(tot[:], ssq[:], ones[:])
        tot_sb = pool.tile([1, 1], F32)
        nc.scalar.activation(out=tot_sb[:], in_=tot[:], func=AF.Copy)
        bc = psum.tile([P, 1], F32)
        nc.tensor.matmul(bc[:], onesr[:], tot_sb[:])
        rec = pool.tile([P, 1], F32)
        nc.vector.reciprocal(out=rec[:], in_=bc[:])
        rstd = pool.tile([P, 1], F32)
        nc.scalar.activation(out=rstd[:], in_=rec[:], func=AF.Sqrt)
        outt = pool.tile([P, T], F32)
        nc.scalar.activation(out=outt[:], in_=gam[:, :, 0], func=AF.Copy,
                             scale=rstd[:])
        nc.sync.dma_start(out=yv[:], in_=outt[:])

    nc.compile()
    return nc


_NC_CACHE = {}

IMPL = os.environ.get("KERNEL_IMPL", "v3")


def _parse_opts():
    """KERNEL_OPTS="n_chunks=6,bufs=2,use_f16=0" -> kwargs for build_v3."""
    opts = {}
    for kv in os.environ.get("KERNEL_OPTS", "").split(","):
        if not kv.strip():
            continue
        k, v = kv.split("=")
        if v in ("0", "1"):
            opts[k.strip()] = bool(int(v))
        elif v.isdigit():
            opts[k.strip()] = int(v)
        else:
            opts[k.strip()] = v.strip()
    return opts


def _get_nc():
    if "nc" not in _NC_CACHE:
        if IMPL == "v1":
            _NC_CACHE["nc"] = build_nc()
        else:
            _NC_CACHE["nc"] = build_v3(**_parse_opts())
    return _NC_CACHE["nc"]


def make_in_maps(feats):
    in_maps = []
    for core in range(NCORES):
        b = core if core < B else 0  # cores >= B chew duplicate data
        in_maps.append({
            "x": np.ascontiguousarray(feats[b]),
            "row0": np.ascontiguousarray(feats[b, 0:1, :]),
        })
    return in_maps


def gather_out(results):
    return np.concatenate([results[b]["y"] for b in range(B)])


def kernel(coords: np.ndarray, features: np.ndarray) -> np.ndarray:
    feats = np.ascontiguousarray(np.asarray(features), dtype=np.float32)
    assert feats.shape == (B, N, C), feats.shape
    nc = _get_nc()
    res = run_bass_kernel_spmd(nc, make_in_maps(feats),
                               core_ids=list(range(NCORES)))
    return gather_out(res.results).astype(np.float32)


# revision 8
# speedup vs baseline: 1.0196x; 1.0196x over previous
"""Trainium2 Bass kernel for nn_JointNet_23785528885377 (retrieval_knn).

Math note: the reference computes nn_idx = argmin(d2, axis=1) over the full
NxN squared-distance matrix but only consumes row 0 of the gathered
neighbors (exp_neighbor = exp(neigh[0]) = exp(f[nn_idx[0]])). Coords are
ints < 100, so d2 is exact integer arithmetic in fp32, d2[0,0] == 0 is the
global minimum of row 0, and argmin tie-breaks to the lowest index =>
nn_idx[0] == 0 for ANY valid input. Hence exp_neighbor == exp(relu(x[0,:]))
and the whole cdist+argmin is dead code. Per cloud:

    f      = relu(x)                               [N,C]
    rowmax = max_c f                               [N]
    gamma  = max_c(f * exp(f) * exp(-f0)) / rowmax [N]   (f0 = relu(x[0,:]))
    out    = gamma / ||gamma||_2

Further algebra used by the v3 kernel (valid whenever a row has at least
one nonnegative channel, which holds for this data — all-negative rows
give NaN in the reference anyway via 0/0):

    relu can be dropped inside the max: for x<0 the term x*exp(x)*e0inv
    is negative while the relu'd term is 0, so it never wins the max.
    rowmax = max_c x = ln(max_c exp(x))  (exp monotone), so one dual
    reduce over [exp(x) | x*e0inv*exp(x)] yields both row stats.
    1/sqrt(s) = exp(-0.5*ln(s)) keeps ACT on two tables (Exp, Ln).

Sharding: one cloud per core (B=2 clouds; cores 2-7 run the same SPMD
program on duplicate data and are ignored). A cross-core AllReduce was
measured at ~56us active time on this runtime — far more than the whole
kernel — so the row-sharded 8-core variant loses; data-parallel it is.
"""

import os

import numpy as np
from contextlib import ExitStack

import concourse.bass as bass
import concourse.bacc as bacc
import concourse.tile as tile
from concourse import mybir
from concourse.bass_utils import run_bass_kernel_spmd

B, N, C = 2, 12288, 32
P = 128
NCORES = 8

AF = mybir.ActivationFunctionType
F32 = mybir.dt.float32
F16 = mybir.dt.float16


def build_v3(n_chunks=4, use_f16=True, dual_queue=True, tot_eng="pe",
             t2_eng="gpsimd", half_first=True):
    """Optimized single-cloud kernel. Empirical constraints of this stack:
    - gpsimd compute silently corrupts when reading ACT-produced tiles and
      hard-crashes on stride-0 APs; safe with DMA/DVE-produced inputs.
    - tensor_tensor_reduce / AP-scalar tensor_scalar don't lower (symbolic
      AP assert); gpsimd free-axis reduces unsupported (axis C only).
    - Only one ACT table stays resident: every Exp<->Sqrt/Ln switch costs
      1.28us, so the kernel uses Exp for the chunks and one Sqrt, whose
      table load is pinned (via a data dependency on the last exp) to
      overlap the DVE drain.
    - DVE reduces run ~1 elem/cycle/lane regardless of dtype; f16 only
      speeds tensor_tensor (2x). DVE is the bottleneck: per chunk it runs
      prod-mul (f16) + 2 row-max reduces; t2 = x*e0inv runs on gpsimd.

    Epilogue is hop-minimized: one DVE block (1/rowmax, gamma, gamma^2,
    row-sum), cross-partition sum via PE (or gpsimd axis-C reduce), DVE
    reciprocal, ACT sqrt, PE broadcast, DVE final scale, DMA out.
    """
    T = N // P
    assert T % n_chunks == 0
    TCH = T // n_chunks

    nc = bacc.Bacc("TRN2", target_bir_lowering=False, debug=False)
    x = nc.dram_tensor("x", [N, C], F32, kind="ExternalInput")
    r0 = nc.dram_tensor("row0", [1, C], F32, kind="ExternalInput")
    y = nc.dram_tensor("y", [N], F32, kind="ExternalOutput")

    xv = x.rearrange("(p t) c -> p t c", p=P)  # [128, 96, 32]
    yv = y.rearrange("(p t) -> p t", p=P)      # [128, 96]

    EDT = F16 if use_f16 else F32

    with tile.TileContext(nc) as tc, ExitStack() as ctx:
        pool = ctx.enter_context(tc.tile_pool(name="main", bufs=1))
        ch = ctx.enter_context(tc.tile_pool(name="chunks", bufs=n_chunks))
        psum = ctx.enter_context(tc.tile_pool(name="psum", bufs=1, space="PSUM"))

        # r0 load first (1 descriptor, lands fast), then all chunk DMA
        # triggers before any ACT op so no trigger queues behind the Exp
        # table load.
        r0row = pool.tile([1, C], F32)
        r0q = nc.scalar if dual_queue else nc.sync
        r0q.dma_start(out=r0row[:], in_=r0[0:1, :])
        xts = []
        for j in range(n_chunks):
            sl = slice(j * TCH, (j + 1) * TCH)
            xt = ch.tile([P, TCH, C], F32, tag=f"xt{j}")
            if j == 0 and half_first and dual_queue:
                # split chunk 0 across both queues so compute ramps sooner
                nc.sync.dma_start(out=xt[0:64], in_=xv[0:64, sl, :])
                nc.scalar.dma_start(out=xt[64:128], in_=xv[64:128, sl, :])
            else:
                dmae = nc.scalar if (dual_queue and j % 2 == 1) else nc.sync
                dmae.dma_start(out=xt[:], in_=xv[:, sl, :])
            xts.append(xt)

        # e0inv = exp(-relu(row0)), broadcast to [P,TCH,C] via PE + DVE copy.
        # high_priority: the whole t2/prod pipeline gates on e0mat, so the
        # scheduler must not slot chunk reduces ahead of this chain.
        with tc.high_priority():
            f0 = pool.tile([1, C], F32)
            nc.vector.tensor_scalar_max(f0[:], r0row[:], 0.0)
            e0row = pool.tile([1, C], F32)
            nc.scalar.activation(out=e0row[:], in_=f0[:], func=AF.Exp,
                                 scale=-1.0)
            onesr = pool.tile([1, P], F32)
            nc.vector.memset(onesr[:], 1.0)
            ones = pool.tile([P, 1], F32)
            nc.vector.memset(ones[:], 1.0)
            e0psum = psum.tile([P, C], F32)
            nc.tensor.matmul(e0psum[:], onesr[:], e0row[:])
            e0mat = pool.tile([P, TCH, C], F32)
            e0p = e0psum[:]
            nc.vector.tensor_copy(
                e0mat[:],
                bass.AP(tensor=e0p.tensor, offset=e0p.offset,
                        ap=[e0p.ap[0], [0, TCH], e0p.ap[1]]),
            )
        t2e = getattr(nc, t2_eng)

        # [:, j, 0, :] = rowmax(x), [:, j, 1, :] = rowmax(x*e0inv*exp(x))
        mm = pool.tile([P, n_chunks, 2, TCH], F32)

        exs = []
        for j in range(n_chunks):
            xt = xts[j]
            ex = ch.tile([P, TCH, C], EDT, tag=f"ex{j}")
            nc.scalar.activation(out=ex[:], in_=xt[:], func=AF.Exp)
            exs.append(ex)
            t2 = ch.tile([P, TCH, C], EDT, tag=f"t2{j}")
            t2e.tensor_mul(t2[:], xt[:], e0mat[:])
            prod = ch.tile([P, TCH, C], EDT, tag=f"prod{j}")
            nc.vector.tensor_mul(prod[:], ex[:], t2[:])
            nc.vector.reduce_max(out=mm[:, j, 1, :], in_=prod[:],
                                 axis=mybir.AxisListType.X)
            nc.vector.reduce_max(out=mm[:, j, 0, :], in_=xt[:],
                                 axis=mybir.AxisListType.X)

        # prefetch the Sqrt table; the read of ex[-1] pins it after the
        # last exp so the load overlaps the DVE drain of chunks 2-3.
        sqdummy = pool.tile([1, 1], F32)
        nc.scalar.activation(out=sqdummy[:], in_=exs[-1][0:1, 0, 0:1],
                             func=AF.Sqrt)

        # ---- epilogue: one DVE block, then the norm scalar chain ----
        rinv = pool.tile([P, T], F32)
        nc.vector.reciprocal(out=rinv[:], in_=mm[:, :, 0, :])
        gam = pool.tile([P, T], F32)
        nc.vector.tensor_mul(gam[:], mm[:, :, 1, :], rinv[:])
        sq = pool.tile([P, T], F32)
        nc.vector.tensor_mul(sq[:], gam[:], gam[:])
        ssq = pool.tile([P, 1], F32)
        nc.vector.reduce_sum(out=ssq[:], in_=sq[:], axis=mybir.AxisListType.X)

        rec = pool.tile([1, 1], F32)
        if tot_eng == "gpsimd":
            tot_sb = pool.tile([1, 1], F32)
            nc.gpsimd.tensor_reduce(out=tot_sb[:], in_=ssq[:],
                                    axis=mybir.AxisListType.C,
                                    op=mybir.AluOpType.add)
            nc.vector.reciprocal(out=rec[:], in_=tot_sb[:])
        else:
            tot = psum.tile([1, 1], F32)
            nc.tensor.matmul(tot[:], ssq[:], ones[:])
            nc.vector.reciprocal(out=rec[:], in_=tot[:])
        rstd = pool.tile([1, 1], F32)
        nc.scalar.activation(out=rstd[:], in_=rec[:], func=AF.Sqrt)
        bc = psum.tile([P, 1], F32)
        nc.tensor.matmul(bc[:], onesr[:], rstd[:])
        bcs = pool.tile([P, 1], F32)
        nc.vector.tensor_copy(bcs[:], bc[:])
        outt = pool.tile([P, T], F32)
        bcsap = bcs[:]
        nc.vector.tensor_mul(
            outt[:], gam[:],
            bass.AP(tensor=bcsap.tensor, offset=bcsap.offset,
                    ap=[bcsap.ap[0], [0, T]]),
        )
        nc.sync.dma_start(out=yv[:], in_=outt[:])

    nc.compile()
    return nc


def build_nc(n_rows=N, n_chunks=4, bufs=2):
    """v1 baseline (kept as fallback): relu/exp on ACT, muls+reduces on DVE."""
    T = n_rows // P
    TCH = T // n_chunks

    nc = bacc.Bacc("TRN2", target_bir_lowering=False, debug=False)
    x = nc.dram_tensor("x", [n_rows, C], F32, kind="ExternalInput")
    r0 = nc.dram_tensor("row0", [1, C], F32, kind="ExternalInput")
    y = nc.dram_tensor("y", [n_rows], F32, kind="ExternalOutput")

    xv = x.rearrange("(p t) c -> p t c", p=P)
    yv = y.rearrange("(p t) -> p t", p=P)

    with tile.TileContext(nc) as tc, ExitStack() as ctx:
        pool = ctx.enter_context(tc.tile_pool(name="main", bufs=1))
        ch = ctx.enter_context(tc.tile_pool(name="chunks", bufs=bufs))
        psum = ctx.enter_context(tc.tile_pool(name="psum", bufs=1, space="PSUM"))

        r0rep = pool.tile([P, C], F32)
        r0ap = r0[0, :]
        nc.sync.dma_start(
            out=r0rep[:],
            in_=bass.AP(tensor=r0ap.tensor, offset=r0ap.offset,
                        ap=[[0, P]] + list(r0ap.ap)),
        )
        e0 = pool.tile([P, C], F32)
        nc.scalar.activation(out=e0[:], in_=r0rep[:], func=AF.Relu)
        nc.scalar.activation(out=e0[:], in_=e0[:], func=AF.Exp, scale=-1.0)
        e0ap = e0[:]

        gam = pool.tile([P, T, 1], F32)
        for j in range(n_chunks):
            sl = slice(j * TCH, (j + 1) * TCH)
            xt = ch.tile([P, TCH, C], F32, tag="xt")
            nc.sync.dma_start(out=xt[:], in_=xv[:, sl, :])
            f = ch.tile([P, TCH, C], F32, tag="f")
            nc.scalar.activation(out=f[:], in_=xt[:], func=AF.Relu)
            rmax = ch.tile([P, TCH, 1], F32, tag="rmax")
            nc.vector.reduce_max(out=rmax[:], in_=f[:], axis=mybir.AxisListType.X)
            ex = ch.tile([P, TCH, C], F32, tag="ex")
            nc.scalar.activation(out=ex[:], in_=f[:], func=AF.Exp)
            nc.vector.tensor_mul(ex[:], ex[:], f[:])
            e0b = bass.AP(tensor=e0ap.tensor, offset=e0ap.offset,
                          ap=[e0ap.ap[0], [0, TCH], e0ap.ap[1]])
            nc.vector.tensor_mul(ex[:], ex[:], e0b)
            m = ch.tile([P, TCH, 1], F32, tag="m")
            nc.vector.reduce_max(out=m[:], in_=ex[:], axis=mybir.AxisListType.X)
            rinv = ch.tile([P, TCH, 1], F32, tag="rinv")
            nc.vector.reciprocal(out=rinv[:], in_=rmax[:])
            nc.vector.tensor_mul(gam[:, sl, :], m[:], rinv[:])

        sq = pool.tile([P, T, 1], F32)
        ssq = pool.tile([P, 1], F32)
        nc.scalar.activation(out=sq[:], in_=gam[:], func=AF.Square,
                             accum_out=ssq[:])
        ones = pool.tile([P, 1], F32)
        nc.vector.memset(ones[:], 1.0)
        onesr = pool.tile([1, P], F32)
        nc.vector.memset(onesr[:], 1.0)
        tot = psum.tile([1, 1], F32)
        nc.tensor.matmul(tot[:], ssq[:], ones[:])
        tot_sb = pool.tile([1, 1], F32)
        nc.scalar.activation(out=tot_sb[:], in_=tot[:], func=AF.Copy)
        bc = psum.tile([P, 1], F32)
        nc.tensor.matmul(bc[:], onesr[:], tot_sb[:])
        rec = pool.tile([P, 1], F32)
        nc.vector.reciprocal(out=rec[:], in_=bc[:])
        rstd = pool.tile([P, 1], F32)
        nc.scalar.activation(out=rstd[:], in_=rec[:], func=AF.Sqrt)
        outt = pool.tile([P, T], F32)
        nc.scalar.activation(out=outt[:], in_=gam[:, :, 0], func=AF.Copy,
                             scale=rstd[:])
        nc.sync.dma_start(out=yv[:], in_=outt[:])

    nc.compile()
    return nc


_NC_CACHE = {}

IMPL = os.environ.get("KERNEL_IMPL", "v3")


def _parse_opts():
    """KERNEL_OPTS="n_chunks=6,bufs=2,use_f16=0" -> kwargs for build_v3."""
    opts = {}
    for kv in os.environ.get("KERNEL_OPTS", "").split(","):
        if not kv.strip():
            continue
        k, v = kv.split("=")
        if v in ("0", "1"):
            opts[k.strip()] = bool(int(v))
        elif v.isdigit():
            opts[k.strip()] = int(v)
        else:
            opts[k.strip()] = v.strip()
    return opts


def _get_nc():
    if "nc" not in _NC_CACHE:
        if IMPL == "v1":
            _NC_CACHE["nc"] = build_nc()
        else:
            _NC_CACHE["nc"] = build_v3(**_parse_opts())
    return _NC_CACHE["nc"]


def make_in_maps(feats):
    in_maps = []
    for core in range(NCORES):
        b = core if core < B else 0  # cores >= B chew duplicate data
        in_maps.append({
            "x": np.ascontiguousarray(feats[b]),
            "row0": np.ascontiguousarray(feats[b, 0:1, :]),
        })
    return in_maps


def gather_out(results):
    return np.concatenate([results[b]["y"] for b in range(B)])


def kernel(coords: np.ndarray, features: np.ndarray) -> np.ndarray:
    feats = np.ascontiguousarray(np.asarray(features), dtype=np.float32)
    assert feats.shape == (B, N, C), feats.shape
    nc = _get_nc()
    res = run_bass_kernel_spmd(nc, make_in_maps(feats),
                               core_ids=list(range(NCORES)))
    return gather_out(res.results).astype(np.float32)
